# revision 3
# baseline (speedup 1.0000x reference)
"""Trainium2 Bass kernel for a 3-layer conditional LSTM (SMILES RNN) with
encoder/decoder feedback.

Scheme "W" (weights-stationary): all gate matmuls keep the WEIGHTS as the
128x128 stationary operand (Fast Weight Load streams them at 2 fp16
elems/cycle/partition -- 2x the moving-port rate the previous kernel paid)
and stream the tiny transposed activations xT [128, 16] as the moving
operand.  Consequences:
  - gates land in PSUM as [gate-on-partition, batch-free] tiles; the per-cell
    PSUM layout is [i0..i3 | f0..f3 | o0..o3 | g0..g3] (16 tiles x 16 cols),
    so pointwise ops are full-lane [128, 64] ACT/DVE instructions.
  - h is produced directly in hT orientation -> zero PE transposes.
  - logits land transposed -> the decoder->encoder feedback (folded through
    the rank-47 logits into a K=128-padded augmented contraction) is a plain
    partition-sliced copy into the xaug moving tile.
  - biases ride as a 9th K-chunk (weight row at partition 0, moving operand =
    a ones-at-partition-0 tile).

Distribution: pure data parallel, batch 128 -> 16 rows per core, weights
replicated; the sequential scan stays core-local (no collectives).
"""

import os
import numpy as np

B, T, H, O, P, NL = 128, 64, 512, 47, 4, 3
G = 4 * H
NCORES = 8
BL = B // NCORES
KA = 128          # padded aug contraction (47 logits + 4 props + 1 ones + pad)
NKT1 = 9          # k-chunks for cells 1/2: 4 input-h + 4 own-h + 1 bias
NKT0 = 5          # k-chunks for cell 0: 4 own-h + 1 aug(+bias folded)
NGT = 16          # gate tiles per cell (G / 128)
NDK = 5           # dec k-chunks: 4 h2 + 1 bias
MM_DT = "float16"

# torch gate order is (i, f, g, o); psum block order is (i, f, o, g)
BLK2TORCH = [0, 1, 3, 2]

_BENCH_R = int(os.environ.get("BENCH_R", "0"))


def _build_nc(t_steps):
    import concourse.mybir as mybir
    import concourse.tile as tile
    from concourse import bacc

    F32 = mybir.dt.float32
    F16 = getattr(mybir.dt, MM_DT)
    ACT = mybir.ActivationFunctionType

    nc = bacc.Bacc(None, target_bir_lowering=False)

    w0_d = nc.dram_tensor("w0", [128, NKT0, G], F16, kind="ExternalInput")
    w1_d = nc.dram_tensor("w1", [128, NKT1, G], F16, kind="ExternalInput")
    w2_d = nc.dram_tensor("w2", [128, NKT1, G], F16, kind="ExternalInput")
    dec_d = nc.dram_tensor("dec", [128, NDK, 128], F16, kind="ExternalInput")
    xaug_d = nc.dram_tensor("xaug0", [128, BL], F16, kind="ExternalInput")
    # hT slots (NL*4 chunks of BL) + e0 ones block (BL)
    init_d = nc.dram_tensor("init", [128, (NL * 4 + 1) * BL], F16, kind="ExternalInput")
    out_d = nc.dram_tensor("out", [128, t_steps * BL], F32, kind="ExternalOutput")

    with tile.TileContext(nc) as tc:
        with (
            tc.tile_pool(name="weights", bufs=1) as wp,
            tc.tile_pool(name="state", bufs=1) as sp,
            tc.tile_pool(name="gact", bufs=2) as hp,
            tc.tile_pool(name="gpool", bufs=4, space="PSUM") as gp,
            tc.tile_pool(name="dpool", bufs=2, space="PSUM") as dp,
        ):
            w0 = wp.tile([128, NKT0, G], F16)
            nc.gpsimd.dma_start(w0[:], w0_d[:])
            w1 = wp.tile([128, NKT1, G], F16)
            nc.gpsimd.dma_start(w1[:], w1_d[:])
            w2 = wp.tile([128, NKT1, G], F16)
            nc.gpsimd.dma_start(w2[:], w2_d[:])
            dec = wp.tile([128, NDK, 128], F16)
            nc.gpsimd.dma_start(dec[:], dec_d[:])

            xaug = sp.tile([128, BL], F16)
            nc.gpsimd.dma_start(xaug[:], xaug_d[:])
            initt = sp.tile([128, (NL * 4 + 1) * BL], F16)
            nc.gpsimd.dma_start(initt[:], init_d[:])
            if _BENCH_R:
                # pristine copies for per-iteration reset
                xaug_p = sp.tile([128, BL], F16)
                nc.gpsimd.dma_start(xaug_p[:], xaug_d[:])
                init_p = sp.tile([128, (NL * 4 + 1) * BL], F16)
                nc.gpsimd.dma_start(init_p[:], init_d[:])
            hist = sp.tile([128, t_steps * BL], F32)
            e0 = initt[:, NL * 4 * BL:(NL * 4 + 1) * BL]  # ones at partition 0
            cs = []
            for l in range(NL):
                c = sp.tile([128, 4 * BL], F32, tag=f"c{l}")
                nc.vector.memset(c[:], 0.0)
                cs.append(c)

            def hT(l):
                return initt[:, l * 4 * BL:(l + 1) * 4 * BL]

            def hT_k(l, k):
                j = (l * 4 + k) * BL
                return initt[:, j:j + BL]

            def emit(psum, wl, ks, rhs_of=None, start=False, stop=False):
                """gate matmuls: for k-chunks `ks`, all 16 gate tiles.
                rhs_of maps k -> moving operand [128, BL].  One accumulation
                group per psum BANK (2KB zero region): only the very first
                matmul starts, only the very last stops."""
                for ki, k in enumerate(ks):
                    rhs = rhs_of(k)
                    for s in range(NGT):
                        nc.tensor.matmul(
                            psum[:, s * BL:(s + 1) * BL],
                            wl[:, k, s * 128:(s + 1) * 128],
                            rhs,
                            start=(start and ki == 0 and s == 0),
                            stop=(stop and ki == len(ks) - 1 and s == NGT - 1),
                        )

            def pointwise(psum, l):
                """[i|f|o|g] blocks of [128, 64] -> c/h update; h written as
                fp16 straight into the hT slots."""
                W4 = 4 * BL
                ga = hp.tile([128, 4 * W4], F32, tag="ga")
                i_ = ga[:, 0 * W4:1 * W4]
                f_ = ga[:, 1 * W4:2 * W4]
                o_ = ga[:, 2 * W4:3 * W4]
                g_ = ga[:, 3 * W4:4 * W4]
                c = cs[l]
                nc.scalar.activation(ga[:, 0:3 * W4], psum[:, 0:3 * W4], ACT.Sigmoid)
                nc.scalar.activation(g_, psum[:, 3 * W4:4 * W4], ACT.Tanh)
                nc.vector.tensor_mul(i_, i_, g_)      # sig(i)*tanh(g)
                nc.vector.tensor_mul(f_, f_, c[:])    # sig(f)*c
                nc.vector.tensor_add(c[:], i_, f_)
                nc.scalar.activation(g_, c[:], ACT.Tanh)
                nc.vector.tensor_mul(hT(l), o_, g_)   # fp16 out = sig(o)*tanh(c)

            # prologue: cell0 own-h matmuls for t=0 (hT0 starts zeroed)
            g0 = gp.tile([128, NGT * BL], F32, tag="g", name="g0_p")
            emit(g0, w0, range(4), lambda k: hT_k(0, k), start=True)

            for t in range(t_steps):
                # (1) cell1 independent: own-h + bias       [dep: hT1(t-1)]
                g1 = gp.tile([128, NGT * BL], F32, tag="g", name=f"g1_{t}")
                emit(g1, w1, range(4, 9), lambda k: e0 if k == 8 else hT_k(1, k - 4),
                     start=True)
                # (2) cell0 aug chunk (logits/props/bias)   [dep: xaug(t)]
                emit(g0, w0, [4], lambda k: xaug[:], stop=True)
                # (3) cell0 pointwise -> hT0(t)
                pointwise(g0, 0)
                # (4) cell2 independent                     [dep: hT2(t-1)]
                g2 = gp.tile([128, NGT * BL], F32, tag="g", name=f"g2_{t}")
                emit(g2, w2, range(4, 9), lambda k: e0 if k == 8 else hT_k(2, k - 4),
                     start=True)
                # (5) cell1 inputs                          [dep: hT0(t)]
                emit(g1, w1, range(4), lambda k: hT_k(0, k), stop=True)
                # (6) cell1 pointwise -> hT1(t)
                pointwise(g1, 1)
                # (7) cell2 inputs                          [dep: hT1(t)]
                emit(g2, w2, range(4), lambda k: hT_k(1, k), stop=True)
                # (8) next step's cell0 own-h               [dep: hT0(t)]
                if t + 1 < t_steps:
                    g0 = gp.tile([128, NGT * BL], F32, tag="g", name=f"g0_{t + 1}")
                    emit(g0, w0, range(4), lambda k: hT_k(0, k), start=True)
                # (9) cell2 pointwise -> hT2(t)
                pointwise(g2, 2)
                # (10) logits = dec @ h2 + dec_b (transposed: [O, BL])
                lps = dp.tile([128, BL], F32, tag="lps")
                for ki in range(NDK):
                    nc.tensor.matmul(lps[:], dec[:, ki, :],
                                     e0 if ki == 4 else hT_k(2, ki),
                                     start=(ki == 0), stop=(ki == 4))
                # (11) history + feedback
                nc.vector.tensor_copy(hist[:, t * BL:(t + 1) * BL], lps[:])
                nc.vector.tensor_copy(xaug[0:O, :], lps[0:O, :])

            nc.sync.dma_start(out_d[:], hist[:])

    nc.compile()
    return nc


def _gate_perm():
    """psum gate-column order -> torch W_cat column indices."""
    cols = np.empty(G, np.int64)
    for s in range(NGT):
        blk, hc = s // 4, s % 4
        tb = BLK2TORCH[blk]
        cols[s * 128:(s + 1) * 128] = 512 * tb + 128 * hc + np.arange(128)
    return cols


def _host_fold(inputs):
    """Fold encoder/decoder/properties/biases into per-core device inputs."""
    ins = {k: np.asarray(v) for k, v in inputs.items()}
    f32 = np.float32
    w_ih0 = ins["w_ih0"].astype(f32)
    w_hh0 = ins["w_hh0"].astype(f32)
    enc_w = ins["enc_w"].astype(f32)
    enc_b = ins["enc_b"].astype(f32)
    dec_w = ins["dec_w"].astype(f32)
    dec_b = ins["dec_b"].astype(f32)
    prop = ins["properties"].astype(f32)
    perm = _gate_perm()

    def chunk_stationary(wT, nk):
        """[K, G] (torch col order) -> [128, nk, G] with permuted gate cols."""
        wTp = wT[:, perm]
        return np.ascontiguousarray(
            wTp.reshape(nk, 128, G).transpose(1, 0, 2))

    # cell0: own-h chunks + K=128-padded aug chunk [logitsT;props;1;pad]
    Wx0 = w_ih0[:, :H]
    Wp0 = w_ih0[:, H:]
    A0 = Wx0 @ enc_w                                   # [G, O]
    b0c = Wx0 @ enc_b + ins["b_ih0"] + ins["b_hh0"]    # [G]
    aug = np.zeros((128, G), f32)
    aug[:O] = A0.T
    aug[O:O + P] = Wp0.T
    aug[O + P] = b0c
    w0 = np.concatenate([chunk_stationary(w_hh0.T.astype(f32), 4),
                         aug[:, perm][:, None, :]], axis=1)  # [128, 5, G]

    def cell_rest(li):
        wih = ins["w_ih_rest"][li].astype(f32)
        whh = ins["w_hh_rest"][li].astype(f32)
        bias = (ins["b_ih_rest"][li] + ins["b_hh_rest"][li]).astype(f32)
        brow = np.zeros((128, G), f32)
        brow[0] = bias
        return np.concatenate([chunk_stationary(wih.T, 4),
                               chunk_stationary(whh.T, 4),
                               brow[:, perm][:, None, :]], axis=1)  # [128, 9, G]

    w1 = cell_rest(0)
    w2 = cell_rest(1)

    decs = np.zeros((128, NDK, 128), f32)
    decT = dec_w.T                                      # [H, O]
    for ki in range(4):
        decs[:, ki, :O] = decT[ki * 128:(ki + 1) * 128]
    decs[0, 4, :O] = dec_b

    init = np.zeros((128, (NL * 4 + 1) * BL), f32)
    init[0, NL * 4 * BL:] = 1.0

    mmdt = np.float16 if MM_DT == "float16" else f32
    shared = {
        "w0": w0.astype(mmdt), "w1": w1.astype(mmdt), "w2": w2.astype(mmdt),
        "dec": np.ascontiguousarray(decs).astype(mmdt),
        "init": np.ascontiguousarray(init).astype(mmdt),
    }
    in_maps = []
    for cid in range(NCORES):
        xaug = np.zeros((128, BL), f32)
        xaug[1, :] = 1.0                               # logits_init = onehot(1)
        xaug[O:O + P, :] = prop[cid * BL:(cid + 1) * BL, :].T
        xaug[O + P, :] = 1.0
        in_maps.append({**shared, "xaug0": np.ascontiguousarray(xaug).astype(mmdt)})
    return in_maps


_NC_CACHE = {}


def _run(inputs, t_steps):
    from concourse.bass_utils import run_bass_kernel_spmd

    if t_steps not in _NC_CACHE:
        _NC_CACHE[t_steps] = _build_nc(t_steps)
    nc = _NC_CACHE[t_steps]
    in_maps = _host_fold(inputs)
    res = run_bass_kernel_spmd(nc, in_maps, core_ids=list(range(NCORES)))
    outs = []
    for cid in range(NCORES):
        hist = res.results[cid]["out"].reshape(128, t_steps, BL)
        outs.append(hist[:O].transpose(2, 1, 0))       # [BL, T, O]
    return np.ascontiguousarray(np.concatenate(outs, axis=0))


def kernel(**inputs):
    t_steps = np.asarray(inputs["x"]).shape[1]
    return _run(inputs, t_steps)
